# revision 5
# baseline (speedup 1.0000x reference)
"""Trainium2 Bass kernel for nn_BinarizeLayer.

out[b, f] = (medians[f] > 0) AND (inputs[b, f] >= medians[f])

Host preprocessing folds the two conditions into one comparison:
m2[f] = medians[f] if medians[f] > 0 else +inf, so out = inputs >= m2
(inputs are finite, so x >= +inf is always False).

Data-parallel over 8 NeuronCores: each core handles a 2048-row slice of
the 16384x8192 f32 input, compares against the replicated median row,
and writes a uint8 0/1 output (4x less store bandwidth than f32).
"""

import numpy as np

import concourse.bacc as bacc
import concourse.mybir as mybir
from concourse import tile
from concourse.bass_utils import run_bass_kernel_spmd

N_CORES = 8
B, F = 16384, 8192
BS = B // N_CORES  # rows per core
P = 128  # SBUF partitions
N_TILES = BS // P  # row-tiles per core


def _build():
    nc = bacc.Bacc(
        "TRN2",
        target_bir_lowering=False,
        debug=False,
        num_devices=N_CORES,
    )
    x = nc.declare_dram_parameter("x", [BS, F], mybir.dt.float32, isOutput=False)
    med = nc.declare_dram_parameter("med", [P, F], mybir.dt.float32, isOutput=False)
    out = nc.declare_dram_parameter("out", [BS, F], mybir.dt.uint8, isOutput=True)

    with tile.TileContext(nc) as tc:
        with (
            tc.tile_pool(name="const", bufs=1) as cpool,
            tc.tile_pool(name="io", bufs=4) as pool,
        ):
            med_t = cpool.tile([P, F], mybir.dt.float32)
            nc.sync.dma_start(out=med_t[:], in_=med[:])
            for i in range(N_TILES):
                xt = pool.tile([P, F], mybir.dt.float32, tag="x")
                nc.sync.dma_start(out=xt[:], in_=x[i * P : (i + 1) * P, :])
                # Write the u8 result in place over the head of the f32
                # tile (write offset trails read offset, so no hazard) —
                # keeps reuse hazards on the DMA, which can carry more
                # sync waits than TensorTensor's ISA encoding.
                xt_u8 = xt.bitcast(mybir.dt.uint8)
                nc.vector.tensor_tensor(
                    xt_u8[:, :F], xt[:], med_t[:], mybir.AluOpType.is_ge
                )
                nc.sync.dma_start(out=out[i * P : (i + 1) * P, :], in_=xt_u8[:, :F])
    nc.compile()
    return nc


def kernel(inputs, medians):
    x = np.ascontiguousarray(np.asarray(inputs, dtype=np.float32))
    m = np.asarray(medians, dtype=np.float32)
    m2 = np.where(m > 0, m, np.float32(np.inf)).astype(np.float32)
    med_rep = np.ascontiguousarray(np.broadcast_to(m2[None, :], (P, F)))

    nc = _build()
    in_maps = [
        {"x": x[c * BS : (c + 1) * BS], "med": med_rep} for c in range(N_CORES)
    ]
    res = run_bass_kernel_spmd(nc, in_maps, list(range(N_CORES))).results
    out = np.concatenate([r["out"] for r in res], axis=0)
    return out.astype(bool)
